# revision 10
# baseline (speedup 1.0000x reference)
"""Cross-attention Trainium2 kernel (8 NeuronCores, SPMD).

Problem: B=4, S=3072, SKV=1036, D_EMBED=1024, D_CROSS=768, H=8, d_head=128.
Sharding: core c -> (batch b = c//2, query-row half sh = c%2). Each core
computes its full [1536, 1024] output slice; gather is pure concatenation.

Per-core device program (all tensors feature-on-partition, token-on-free):
  qT[d,s]  = WqT.T @ xT    (fp32r matmuls, bias bq via ACT-copy bias)
  kT[d,t]  = WkT.T @ yT    (bias bk via ACT-copy bias)
  v[t,d]   = yT.T @ WvT    (-> bf16; bias bv folded into output const)
  per head h, per 512-col s-chunk:
    scores[s,t] = qT_h_slice.T @ kT_h      (psum, fp32r)
    p = exp(scores * 1/sqrt(128))          (ACT, accum_out -> row sums)
    p *= 1/rowsum                          (DVE tensor_scalar, bf16)
    pT = transpose(p)                      (PE transposes, bf16)
    attnT_h[:, sc] = sum_t v_h[t].T @ pT[t]  (psum accumulate)
  finalT[e,s] = WoT.T @ attnT + const      (fp32r; const = Wo@bv + bo)
  out[s,e]   = transpose(finalT)           (PE transposes, fp32r)
"""

import sys

sys.path.insert(0, "/opt/trn_rl_repo")

import math

import numpy as np

import concourse.bass as bass
import concourse.mybir as mybir
import concourse.tile as tile
from concourse import bacc
from concourse.bass import ts, ds
from concourse.bass_utils import run_bass_kernel_spmd
from concourse.masks import make_identity
from concourse import bass_isa

N_CORES = 8
B, S, SKV = 4, 3072, 1036
DE, DC, H, DH = 1024, 768, 8, 128
S_SH = S // 2          # 1536 query rows per core
NS = S_SH // 128       # 12 s-tiles
NSC = S_SH // 512      # 3 s-chunks of 512
NT_FULL = SKV // 128   # 8 full t-tiles
T_REM = SKV - NT_FULL * 128  # 12
NT = NT_FULL + 1       # 9 t-tiles
NE = DE // 128         # 8
NCC = DC // 128        # 6
INV_SQRT_DH = 1.0 / math.sqrt(DH)

F32 = mybir.dt.float32
F32R = mybir.dt.float32r
BF16 = mybir.dt.bfloat16


def _t_width(ti):
    return 128 if ti < NT_FULL else T_REM


def build_bass():
    nc = bacc.Bacc("TRN2", target_bir_lowering=False, debug=False)

    xT_d = nc.dram_tensor("xT", [DE, S_SH], F32, kind="ExternalInput").ap()
    yT_d = nc.dram_tensor("yT", [DC, SKV], F32, kind="ExternalInput").ap()
    wqT_d = nc.dram_tensor("WqT", [DE, DE], F32, kind="ExternalInput").ap()
    wkT_d = nc.dram_tensor("WkT", [DC, DE], F32, kind="ExternalInput").ap()
    wvT_d = nc.dram_tensor("WvT", [DC, DE], F32, kind="ExternalInput").ap()
    woT_d = nc.dram_tensor("WoT", [DE, DE], F32, kind="ExternalInput").ap()
    bq_d = nc.dram_tensor("bq", [DE], F32, kind="ExternalInput").ap()
    bk_d = nc.dram_tensor("bk", [DE], F32, kind="ExternalInput").ap()
    cst_d = nc.dram_tensor("constb", [DE], F32, kind="ExternalInput").ap()
    out_d = nc.dram_tensor("out", [S_SH, DE], F32, kind="ExternalOutput").ap()

    with tile.TileContext(nc) as tc:
        with (
            tc.tile_pool(name="misc", bufs=1) as misc,
            tc.tile_pool(name="attn_keep", bufs=1) as attn_keep,
        ):
            ident_f = misc.tile([128, 128], F32)
            make_identity(nc, ident_f)
            bq_sb = misc.tile([128, NE], F32)
            nc.sync.dma_start(bq_sb, bq_d.rearrange("(j p) -> p j", p=128))
            bk_sb = misc.tile([128, NE], F32)
            nc.sync.dma_start(bk_sb, bk_d.rearrange("(j p) -> p j", p=128))
            cst_sb = misc.tile([128, NE], F32)
            nc.sync.dma_start(cst_sb, cst_d.rearrange("(j p) -> p j", p=128))

            attnT = attn_keep.tile([128, NE, S_SH], F32R)

            # ---- stage Q: qT[d, s] ----
            with tc.tile_pool(name="q_keep", bufs=1) as q_keep:
                qT = q_keep.tile([128, NE, S_SH], F32R)
                with (
                    tc.tile_pool(name="xq_in", bufs=1) as xq_in,
                    tc.tile_pool(name="ps_q", bufs=8, space="PSUM") as ps_q,
                ):
                    xT = xq_in.tile([128, NE, S_SH], F32R)
                    wqT = xq_in.tile([128, NE, DE], F32R)
                    for i in range(NE):
                        nc.gpsimd.dma_start(xT[:, i], xT_d[ts(i, 128)])
                        nc.gpsimd.dma_start(wqT[:, i], wqT_d[ts(i, 128)])
                    for di in range(NE):
                        for sc in range(NSC):
                            ps = ps_q.tile([128, 512], F32, tag="psq")
                            for ei in range(NE):
                                nc.tensor.matmul(
                                    ps,
                                    wqT[:, ei, ts(di, 128)],
                                    xT[:, ei, ts(sc, 512)],
                                    start=(ei == 0),
                                    stop=(ei == NE - 1),
                                )
                            nc.scalar.activation(
                                qT[:, di, ts(sc, 512)], ps,
                                mybir.ActivationFunctionType.Identity,
                                bias=bq_sb[:, ds(di, 1)],
                            )

                # ---- stage K/V ----
                kv_keep = tc.alloc_tile_pool(name="kv_keep", bufs=1)
                kT = kv_keep.tile([128, NE, SKV], F32R)
                v_sb = kv_keep.tile([128, NT, DE], BF16)
                with tc.tile_pool(name="y_in", bufs=1) as y_in:
                    yT = y_in.tile([128, NCC, SKV], F32R)
                    for i in range(NCC):
                        nc.gpsimd.dma_start(yT[:, i], yT_d[ts(i, 128)])

                    with (
                        tc.tile_pool(name="wk_in", bufs=1) as wk_in,
                        tc.tile_pool(name="ps_k", bufs=8, space="PSUM") as ps_k,
                    ):
                        wkT = wk_in.tile([128, NCC, DE], F32R)
                        for i in range(NCC):
                            nc.gpsimd.dma_start(wkT[:, i], wkT_d[ts(i, 128)])
                        for di in range(NE):
                            for tc_i in range(3):
                                t0, tw = tc_i * 512, min(512, SKV - tc_i * 512)
                                ps = ps_k.tile([128, 512], F32, tag="psk")
                                for ci in range(NCC):
                                    nc.tensor.matmul(
                                        ps[:, :tw],
                                        wkT[:, ci, ts(di, 128)],
                                        yT[:, ci, ds(t0, tw)],
                                        start=(ci == 0),
                                        stop=(ci == NCC - 1),
                                    )
                                nc.scalar.activation(
                                    kT[:, di, ds(t0, tw)], ps[:, :tw],
                                    mybir.ActivationFunctionType.Identity,
                                    bias=bk_sb[:, ds(di, 1)],
                                )

                    with (
                        tc.tile_pool(name="wv_in", bufs=1) as wv_in,
                        tc.tile_pool(name="ps_v", bufs=8, space="PSUM") as ps_v,
                    ):
                        wvT = wv_in.tile([128, NCC, DE], F32R)
                        for i in range(NCC):
                            nc.gpsimd.dma_start(wvT[:, i], wvT_d[ts(i, 128)])
                        for ti in range(NT):
                            tw = _t_width(ti)
                            for dc in range(2):
                                ps = ps_v.tile([128, 512], F32, tag="psv")
                                for ci in range(NCC):
                                    nc.tensor.matmul(
                                        ps[:tw],
                                        yT[:, ci, ds(ti * 128, tw)],
                                        wvT[:, ci, ts(dc, 512)],
                                        start=(ci == 0),
                                        stop=(ci == NCC - 1),
                                    )
                                nc.vector.tensor_copy(
                                    v_sb[:tw, ti, ts(dc, 512)], ps[:tw]
                                )

                # ---- attention (transposed scores layout) ----
                with (
                    tc.tile_pool(name="awork", bufs=2) as awork,
                    tc.tile_pool(name="ps_s", bufs=4, space="PSUM") as ps_s,
                    tc.tile_pool(name="ps_o", bufs=3, space="PSUM") as ps_o,
                ):
                    for h in range(H):
                        for sc in range(NSC):
                            expT = awork.tile([128, NT, 512], BF16, tag="expT")
                            # scoresT[t, s] = kT_h_tile.T @ qT_h_chunk; exp
                            for ti in range(NT):
                                tw = _t_width(ti)
                                ps = ps_s.tile([128, 512], F32, tag="pss")
                                nc.tensor.matmul(
                                    ps[:tw],
                                    kT[:, h, ds(ti * 128, tw)],
                                    qT[:, h, ts(sc, 512)],
                                    start=True,
                                    stop=True,
                                )
                                nc.scalar.activation(
                                    expT[:tw, ti], ps[:tw],
                                    mybir.ActivationFunctionType.Exp,
                                    scale=INV_SQRT_DH,
                                )
                                if ti == NT - 1:
                                    e8f = awork.tile([T_REM, 512], F32, tag="e8f")
                                    nc.scalar.activation(
                                        e8f, ps[:tw],
                                        mybir.ActivationFunctionType.Exp,
                                        scale=INV_SQRT_DH,
                                    )
                            # attnT_h[:, sc] = sum_t v_h[t].T @ expT[t]  (unnormalized)
                            pso = ps_o.tile([128, 512], F32, tag="pso")
                            for ti in range(NT):
                                tw = _t_width(ti)
                                nc.tensor.matmul(
                                    pso,
                                    v_sb[:tw, ti, ds(h * 128, 128)],
                                    expT[:tw, ti],
                                    start=(ti == 0),
                                    stop=(ti == NT - 1),
                                )
                            nc.scalar.activation(
                                attnT[:, h, ts(sc, 512)], pso,
                                mybir.ActivationFunctionType.Copy,
                            )
                            # Z[s] = sum_t exp: DVE pairwise tree + gpsimd
                            # partition all-reduce; normalize attnT in place
                            s01 = awork.tile([128, 512], F32, tag="s01")
                            s23 = awork.tile([128, 512], F32, tag="s23")
                            s45 = awork.tile([128, 512], F32, tag="s45")
                            s67 = awork.tile([128, 512], F32, tag="s67")
                            nc.vector.tensor_tensor(s01, expT[:, 0], expT[:, 1], op=mybir.AluOpType.add)
                            nc.vector.tensor_tensor(s23, expT[:, 2], expT[:, 3], op=mybir.AluOpType.add)
                            nc.vector.tensor_tensor(s45, expT[:, 4], expT[:, 5], op=mybir.AluOpType.add)
                            nc.vector.tensor_tensor(s67, expT[:, 6], expT[:, 7], op=mybir.AluOpType.add)
                            nc.vector.tensor_tensor(s01, s01, s23, op=mybir.AluOpType.add)
                            nc.vector.tensor_tensor(s45, s45, s67, op=mybir.AluOpType.add)
                            nc.vector.tensor_tensor(s01, s01, s45, op=mybir.AluOpType.add)
                            nc.vector.tensor_tensor(s01[:T_REM], s01[:T_REM], e8f, op=mybir.AluOpType.add)
                            zf = awork.tile([128, 512], F32, tag="zf")
                            nc.gpsimd.partition_all_reduce(
                                zf, s01, channels=128,
                                reduce_op=bass_isa.ReduceOp.add,
                            )
                            zr1 = awork.tile([1, 512], F32, tag="zr1")
                            nc.vector.reciprocal(zr1, zf[:1])
                            zrb = awork.tile([128, 512], F32, tag="zrb")
                            nc.gpsimd.partition_broadcast(zrb, zr1)
                            nc.vector.tensor_tensor(
                                attnT[:, h, ts(sc, 512)],
                                attnT[:, h, ts(sc, 512)], zrb,
                                op=mybir.AluOpType.mult,
                            )
                kv_keep.release()

            # ---- output projection + final transpose ----
            with (
                tc.tile_pool(name="o_in", bufs=1) as o_in,
                tc.tile_pool(name="o_out", bufs=3) as o_out,
                tc.tile_pool(name="ps_f", bufs=4, space="PSUM") as ps_f,
                tc.tile_pool(name="ps_ft", bufs=2, space="PSUM") as ps_ft,
            ):
                woT = o_in.tile([128, NE, DE], F32R)
                for i in range(NE):
                    nc.gpsimd.dma_start(woT[:, i], woT_d[ts(i, 128)])
                finalT = o_in.tile([128, NE, S_SH], F32)
                for ei in range(NE):
                    for sc in range(NSC):
                        ps = ps_f.tile([128, 512], F32, tag="psf")
                        for di in range(NE):
                            nc.tensor.matmul(
                                ps,
                                woT[:, di, ts(ei, 128)],
                                attnT[:, di, ts(sc, 512)],
                                start=(di == 0),
                                stop=(di == NE - 1),
                            )
                        nc.scalar.activation(
                            finalT[:, ei, ts(sc, 512)], ps,
                            mybir.ActivationFunctionType.Identity,
                            bias=cst_sb[:, ds(ei, 1)],
                        )
                for si in range(NS):
                    out_sb = o_out.tile([128, DE], F32, tag="osb")
                    for g in range(2):
                        pst = ps_ft.tile([128, 512], F32, tag="psft")
                        for j in range(4):
                            nc.tensor.transpose(
                                pst[:, ts(j, 128)],
                                finalT[:, g * 4 + j, ts(si, 128)],
                                ident_f,
                            )
                        nc.vector.tensor_copy(out_sb[:, ts(g, 512)], pst)
                    nc.sync.dma_start(out_d[ts(si, 128)], out_sb)

    nc.compile()
    return nc


_NC_CACHE = None


def _get_nc():
    global _NC_CACHE
    if _NC_CACHE is None:
        _NC_CACHE = build_bass()
    return _NC_CACHE


def make_in_maps(inputs):
    x = np.asarray(inputs["x"], np.float32)
    y = np.asarray(inputs["y"], np.float32)
    Wq = np.asarray(inputs["Wq"], np.float32)
    Wk = np.asarray(inputs["Wk"], np.float32)
    Wv = np.asarray(inputs["Wv"], np.float32)
    Wo = np.asarray(inputs["Wo"], np.float32)
    bq = np.asarray(inputs["bq"], np.float32)
    bk = np.asarray(inputs["bk"], np.float32)
    bv = np.asarray(inputs["bv"], np.float32)
    bo = np.asarray(inputs["bo"], np.float32)

    shared = {
        "WqT": np.ascontiguousarray(Wq.T),
        "WkT": np.ascontiguousarray(Wk.T),
        "WvT": np.ascontiguousarray(Wv.T),
        "WoT": np.ascontiguousarray(Wo.T),
        "bq": bq,
        "bk": bk,
        "constb": (Wo @ bv + bo).astype(np.float32),
    }
    in_maps = []
    for c in range(N_CORES):
        b, sh = c // 2, c % 2
        in_maps.append(
            {
                "xT": np.ascontiguousarray(x[b, sh * S_SH : (sh + 1) * S_SH].T),
                "yT": np.ascontiguousarray(y[b].T),
                **shared,
            }
        )
    return in_maps


def gather(results):
    out = np.empty((B, S, DE), np.float32)
    for c in range(N_CORES):
        b, sh = c // 2, c % 2
        out[b, sh * S_SH : (sh + 1) * S_SH] = results[c]["out"]
    return out


def kernel(**inputs) -> np.ndarray:
    nc = _get_nc()
    in_maps = make_in_maps(inputs)
    res = run_bass_kernel_spmd(nc, in_maps, core_ids=list(range(N_CORES)))
    return gather(res.results)


# revision 11
# speedup vs baseline: 1.2002x; 1.2002x over previous
"""Cross-attention Trainium2 kernel (8 NeuronCores, SPMD).

Problem: B=4, S=3072, SKV=1036, D_EMBED=1024, D_CROSS=768, H=8, d_head=128.
Sharding: core c -> (batch b = c//2, query-row half sh = c%2). Each core
computes its full [1536, 1024] output slice; gather is pure concatenation.

Per-core device program (all tensors feature-on-partition, token-on-free):
  qT[d,s]  = WqT.T @ xT    (fp32r matmuls, bias bq via ACT-copy bias)
  kT[d,t]  = WkT.T @ yT    (bias bk via ACT-copy bias)
  v[t,d]   = yT.T @ WvT    (-> bf16; bias bv folded into output const)
  per head h, per 512-col s-chunk:
    scores[s,t] = qT_h_slice.T @ kT_h      (psum, fp32r)
    p = exp(scores * 1/sqrt(128))          (ACT, accum_out -> row sums)
    p *= 1/rowsum                          (DVE tensor_scalar, bf16)
    pT = transpose(p)                      (PE transposes, bf16)
    attnT_h[:, sc] = sum_t v_h[t].T @ pT[t]  (psum accumulate)
  finalT[e,s] = WoT.T @ attnT + const      (fp32r; const = Wo@bv + bo)
  out[s,e]   = transpose(finalT)           (PE transposes, fp32r)
"""

import sys

sys.path.insert(0, "/opt/trn_rl_repo")

import math

import numpy as np

import concourse.bass as bass
import concourse.mybir as mybir
import concourse.tile as tile
from concourse import bacc
from concourse.bass import ts, ds
from concourse.bass_utils import run_bass_kernel_spmd
from concourse.masks import make_identity
from concourse import bass_isa

N_CORES = 8
B, S, SKV = 4, 3072, 1036
DE, DC, H, DH = 1024, 768, 8, 128
S_SH = S // 2          # 1536 query rows per core
NS = S_SH // 128       # 12 s-tiles
NSC = S_SH // 512      # 3 s-chunks of 512
NT_FULL = SKV // 128   # 8 full t-tiles
T_REM = SKV - NT_FULL * 128  # 12
NT = NT_FULL + 1       # 9 t-tiles
NE = DE // 128         # 8
NCC = DC // 128        # 6
INV_SQRT_DH = 1.0 / math.sqrt(DH)

F32 = mybir.dt.float32
F32R = mybir.dt.float32r
BF16 = mybir.dt.bfloat16


def _t_width(ti):
    return 128 if ti < NT_FULL else T_REM


def build_bass():
    nc = bacc.Bacc("TRN2", target_bir_lowering=False, debug=False)

    xT_d = nc.dram_tensor("xT", [DE, S_SH], F32, kind="ExternalInput").ap()
    yT_d = nc.dram_tensor("yT", [DC, SKV], F32, kind="ExternalInput").ap()
    wqT_d = nc.dram_tensor("WqT", [DE, DE], F32, kind="ExternalInput").ap()
    wkT_d = nc.dram_tensor("WkT", [DC, DE], F32, kind="ExternalInput").ap()
    wvT_d = nc.dram_tensor("WvT", [DC, DE], F32, kind="ExternalInput").ap()
    woT_d = nc.dram_tensor("WoT", [DE, DE], F32, kind="ExternalInput").ap()
    bq_d = nc.dram_tensor("bq", [DE], F32, kind="ExternalInput").ap()
    bk_d = nc.dram_tensor("bk", [DE], F32, kind="ExternalInput").ap()
    cst_d = nc.dram_tensor("constb", [DE], F32, kind="ExternalInput").ap()
    out_d = nc.dram_tensor("out", [S_SH, DE], F32, kind="ExternalOutput").ap()

    with tile.TileContext(nc) as tc:
        with (
            tc.tile_pool(name="misc", bufs=1) as misc,
            tc.tile_pool(name="attn_keep", bufs=1) as attn_keep,
        ):
            ident_f = misc.tile([128, 128], F32)
            make_identity(nc, ident_f)
            bq_sb = misc.tile([128, NE], F32)
            nc.sync.dma_start(bq_sb, bq_d.rearrange("(j p) -> p j", p=128))
            bk_sb = misc.tile([128, NE], F32)
            nc.sync.dma_start(bk_sb, bk_d.rearrange("(j p) -> p j", p=128))
            cst_sb = misc.tile([128, NE], F32)
            nc.sync.dma_start(cst_sb, cst_d.rearrange("(j p) -> p j", p=128))

            attnT = attn_keep.tile([128, NE, S_SH], F32R)

            # ---- stage Q: qT[d, s] ----
            with tc.tile_pool(name="q_keep", bufs=1) as q_keep:
                qT = q_keep.tile([128, NE, S_SH], F32R)
                with (
                    tc.tile_pool(name="xq_in", bufs=1) as xq_in,
                    tc.tile_pool(name="ps_q", bufs=8, space="PSUM") as ps_q,
                ):
                    xT = xq_in.tile([128, NE, S_SH], F32R)
                    wqT = xq_in.tile([128, NE, DE], F32R)
                    for i in range(NE):
                        nc.gpsimd.dma_start(xT[:, i], xT_d[ts(i, 128)])
                        nc.gpsimd.dma_start(wqT[:, i], wqT_d[ts(i, 128)])
                    for di in range(NE):
                        for sc in range(NSC):
                            ps = ps_q.tile([128, 512], F32, tag="psq")
                            for ei in range(NE):
                                nc.tensor.matmul(
                                    ps,
                                    wqT[:, ei, ts(di, 128)],
                                    xT[:, ei, ts(sc, 512)],
                                    start=(ei == 0),
                                    stop=(ei == NE - 1),
                                )
                            nc.scalar.activation(
                                qT[:, di, ts(sc, 512)], ps,
                                mybir.ActivationFunctionType.Identity,
                                bias=bq_sb[:, ds(di, 1)],
                            )

                # ---- stage K/V ----
                kv_keep = tc.alloc_tile_pool(name="kv_keep", bufs=1)
                kT = kv_keep.tile([128, NE, SKV], F32R)
                v_sb = kv_keep.tile([128, NT, DE], BF16)
                with tc.tile_pool(name="y_in", bufs=1) as y_in:
                    yT = y_in.tile([128, NCC, SKV], F32R)
                    for i in range(NCC):
                        nc.gpsimd.dma_start(yT[:, i], yT_d[ts(i, 128)])

                    with (
                        tc.tile_pool(name="wk_in", bufs=1) as wk_in,
                        tc.tile_pool(name="ps_k", bufs=8, space="PSUM") as ps_k,
                    ):
                        wkT = wk_in.tile([128, NCC, DE], F32R)
                        for i in range(NCC):
                            nc.gpsimd.dma_start(wkT[:, i], wkT_d[ts(i, 128)])
                        for di in range(NE):
                            for tc_i in range(3):
                                t0, tw = tc_i * 512, min(512, SKV - tc_i * 512)
                                ps = ps_k.tile([128, 512], F32, tag="psk")
                                for ci in range(NCC):
                                    nc.tensor.matmul(
                                        ps[:, :tw],
                                        wkT[:, ci, ts(di, 128)],
                                        yT[:, ci, ds(t0, tw)],
                                        start=(ci == 0),
                                        stop=(ci == NCC - 1),
                                    )
                                nc.scalar.activation(
                                    kT[:, di, ds(t0, tw)], ps[:, :tw],
                                    mybir.ActivationFunctionType.Identity,
                                    bias=bk_sb[:, ds(di, 1)],
                                )

                    with (
                        tc.tile_pool(name="wv_in", bufs=1) as wv_in,
                        tc.tile_pool(name="ps_v", bufs=8, space="PSUM") as ps_v,
                    ):
                        wvT = wv_in.tile([128, NCC, DE], F32R)
                        for i in range(NCC):
                            nc.gpsimd.dma_start(wvT[:, i], wvT_d[ts(i, 128)])
                        for dc in range(2):
                            for ti in range(NT):
                                tw = _t_width(ti)
                                ps = ps_v.tile([128, 512], F32, tag="psv")
                                for ci in range(NCC):
                                    nc.tensor.matmul(
                                        ps[:tw],
                                        yT[:, ci, ds(ti * 128, tw)],
                                        wvT[:, ci, ts(dc, 512)],
                                        start=(ci == 0),
                                        stop=(ci == NCC - 1),
                                    )
                                nc.vector.tensor_copy(
                                    v_sb[:tw, ti, ts(dc, 512)], ps[:tw]
                                )

                # ---- attention (transposed scores layout) ----
                with (
                    tc.tile_pool(name="awork", bufs=2) as awork,
                    tc.tile_pool(name="ps_s", bufs=4, space="PSUM") as ps_s,
                    tc.tile_pool(name="ps_o", bufs=3, space="PSUM") as ps_o,
                ):
                    def stage_scores(h, sc):
                        expT = awork.tile([128, NT, 512], BF16, tag="expT")
                        e8f = awork.tile([T_REM, 512], F32, tag="e8f")
                        for ti in range(NT):
                            tw = _t_width(ti)
                            ps = ps_s.tile([128, 512], F32, tag="pss")
                            nc.tensor.matmul(
                                ps[:tw],
                                kT[:, h, ds(ti * 128, tw)],
                                qT[:, h, ts(sc, 512)],
                                start=True,
                                stop=True,
                            )
                            nc.scalar.activation(
                                expT[:tw, ti], ps[:tw],
                                mybir.ActivationFunctionType.Exp,
                                scale=INV_SQRT_DH,
                            )
                            if ti == NT - 1:
                                nc.scalar.activation(
                                    e8f, ps[:tw],
                                    mybir.ActivationFunctionType.Exp,
                                    scale=INV_SQRT_DH,
                                )
                        return expT, e8f

                    def stage_pv(h, sc, expT, e8f):
                        pso = ps_o.tile([128, 512], F32, tag="pso")
                        for ti in range(NT):
                            tw = _t_width(ti)
                            nc.tensor.matmul(
                                pso,
                                v_sb[:tw, ti, ds(h * 128, 128)],
                                expT[:tw, ti],
                                start=(ti == 0),
                                stop=(ti == NT - 1),
                            )
                        nc.scalar.activation(
                            attnT[:, h, ts(sc, 512)], pso,
                            mybir.ActivationFunctionType.Copy,
                        )
                        s01 = awork.tile([128, 512], F32, tag="s01")
                        s23 = awork.tile([128, 512], F32, tag="s23")
                        s45 = awork.tile([128, 512], F32, tag="s45")
                        s67 = awork.tile([128, 512], F32, tag="s67")
                        nc.vector.tensor_tensor(s01, expT[:, 0], expT[:, 1], op=mybir.AluOpType.add)
                        nc.vector.tensor_tensor(s23, expT[:, 2], expT[:, 3], op=mybir.AluOpType.add)
                        nc.vector.tensor_tensor(s45, expT[:, 4], expT[:, 5], op=mybir.AluOpType.add)
                        nc.vector.tensor_tensor(s67, expT[:, 6], expT[:, 7], op=mybir.AluOpType.add)
                        nc.vector.tensor_tensor(s01, s01, s23, op=mybir.AluOpType.add)
                        nc.vector.tensor_tensor(s45, s45, s67, op=mybir.AluOpType.add)
                        nc.vector.tensor_tensor(s01, s01, s45, op=mybir.AluOpType.add)
                        nc.vector.tensor_tensor(s01[:T_REM], s01[:T_REM], e8f, op=mybir.AluOpType.add)
                        zf = awork.tile([128, 512], F32, tag="zf")
                        nc.gpsimd.partition_all_reduce(
                            zf, s01, channels=128,
                            reduce_op=bass_isa.ReduceOp.add,
                        )
                        zrb = awork.tile([128, 512], F32, tag="zrb")
                        nc.vector.reciprocal_approx_fast(zrb, zf)
                        nc.vector.tensor_tensor(
                            attnT[:, h, ts(sc, 512)],
                            attnT[:, h, ts(sc, 512)], zrb,
                            op=mybir.AluOpType.mult,
                        )

                    iters = [(h, sc) for h in range(H) for sc in range(NSC)]
                    prev = None
                    for h, sc in iters:
                        cur = (h, sc, *stage_scores(h, sc))
                        if prev is not None:
                            stage_pv(*prev)
                        prev = cur
                    stage_pv(*prev)
                kv_keep.release()

            # ---- output projection + final transpose ----
            with (
                tc.tile_pool(name="o_in", bufs=1) as o_in,
                tc.tile_pool(name="o_out", bufs=3) as o_out,
                tc.tile_pool(name="ps_f", bufs=4, space="PSUM") as ps_f,
                tc.tile_pool(name="ps_ft", bufs=2, space="PSUM") as ps_ft,
            ):
                woT = o_in.tile([128, NE, DE], F32R)
                for i in range(NE):
                    nc.gpsimd.dma_start(woT[:, i], woT_d[ts(i, 128)])
                finalT = o_in.tile([128, NE, S_SH], F32)
                for ei in range(NE):
                    for sc in range(NSC):
                        ps = ps_f.tile([128, 512], F32, tag="psf")
                        for di in range(NE):
                            nc.tensor.matmul(
                                ps,
                                woT[:, di, ts(ei, 128)],
                                attnT[:, di, ts(sc, 512)],
                                start=(di == 0),
                                stop=(di == NE - 1),
                            )
                        nc.scalar.activation(
                            finalT[:, ei, ts(sc, 512)], ps,
                            mybir.ActivationFunctionType.Identity,
                            bias=cst_sb[:, ds(ei, 1)],
                        )
                for si in range(NS):
                    out_sb = o_out.tile([128, DE], F32, tag="osb")
                    for g in range(2):
                        pst = ps_ft.tile([128, 512], F32, tag="psft")
                        for j in range(4):
                            nc.tensor.transpose(
                                pst[:, ts(j, 128)],
                                finalT[:, g * 4 + j, ts(si, 128)],
                                ident_f,
                            )
                        nc.vector.tensor_copy(out_sb[:, ts(g, 512)], pst)
                    nc.sync.dma_start(out_d[ts(si, 128)], out_sb)

    nc.compile()
    return nc


_NC_CACHE = None


def _get_nc():
    global _NC_CACHE
    if _NC_CACHE is None:
        _NC_CACHE = build_bass()
    return _NC_CACHE


def make_in_maps(inputs):
    x = np.asarray(inputs["x"], np.float32)
    y = np.asarray(inputs["y"], np.float32)
    Wq = np.asarray(inputs["Wq"], np.float32)
    Wk = np.asarray(inputs["Wk"], np.float32)
    Wv = np.asarray(inputs["Wv"], np.float32)
    Wo = np.asarray(inputs["Wo"], np.float32)
    bq = np.asarray(inputs["bq"], np.float32)
    bk = np.asarray(inputs["bk"], np.float32)
    bv = np.asarray(inputs["bv"], np.float32)
    bo = np.asarray(inputs["bo"], np.float32)

    shared = {
        "WqT": np.ascontiguousarray(Wq.T),
        "WkT": np.ascontiguousarray(Wk.T),
        "WvT": np.ascontiguousarray(Wv.T),
        "WoT": np.ascontiguousarray(Wo.T),
        "bq": bq,
        "bk": bk,
        "constb": (Wo @ bv + bo).astype(np.float32),
    }
    in_maps = []
    for c in range(N_CORES):
        b, sh = c // 2, c % 2
        in_maps.append(
            {
                "xT": np.ascontiguousarray(x[b, sh * S_SH : (sh + 1) * S_SH].T),
                "yT": np.ascontiguousarray(y[b].T),
                **shared,
            }
        )
    return in_maps


def gather(results):
    out = np.empty((B, S, DE), np.float32)
    for c in range(N_CORES):
        b, sh = c // 2, c % 2
        out[b, sh * S_SH : (sh + 1) * S_SH] = results[c]["out"]
    return out


def kernel(**inputs) -> np.ndarray:
    nc = _get_nc()
    in_maps = make_in_maps(inputs)
    res = run_bass_kernel_spmd(nc, in_maps, core_ids=list(range(N_CORES)))
    return gather(res.results)


# revision 13
# speedup vs baseline: 1.2125x; 1.0103x over previous
"""Cross-attention Trainium2 kernel (8 NeuronCores, SPMD).

Problem: B=4, S=3072, SKV=1036, D_EMBED=1024, D_CROSS=768, H=8, d_head=128.
Sharding: core c -> (batch b = c//2, query-row half sh = c%2). Each core
computes its full [1536, 1024] output slice; gather is pure concatenation.

Per-core device program (all tensors feature-on-partition, token-on-free):
  qT[d,s]  = WqT.T @ xT    (fp32r matmuls, bias bq via ACT-copy bias)
  kT[d,t]  = WkT.T @ yT    (bias bk via ACT-copy bias)
  v[t,d]   = yT.T @ WvT    (-> bf16; bias bv folded into output const)
  per head h, per 512-col s-chunk:
    scores[s,t] = qT_h_slice.T @ kT_h      (psum, fp32r)
    p = exp(scores * 1/sqrt(128))          (ACT, accum_out -> row sums)
    p *= 1/rowsum                          (DVE tensor_scalar, bf16)
    pT = transpose(p)                      (PE transposes, bf16)
    attnT_h[:, sc] = sum_t v_h[t].T @ pT[t]  (psum accumulate)
  finalT[e,s] = WoT.T @ attnT + const      (fp32r; const = Wo@bv + bo)
  out[s,e]   = transpose(finalT)           (PE transposes, fp32r)
"""

import sys

sys.path.insert(0, "/opt/trn_rl_repo")

import math

import numpy as np

import concourse.bass as bass
import concourse.mybir as mybir
import concourse.tile as tile
from concourse import bacc
from concourse.bass import ts, ds
from concourse.bass_utils import run_bass_kernel_spmd
from concourse.masks import make_identity
from concourse import bass_isa

N_CORES = 8
B, S, SKV = 4, 3072, 1036
DE, DC, H, DH = 1024, 768, 8, 128
S_SH = S // 2          # 1536 query rows per core
NS = S_SH // 128       # 12 s-tiles
NSC = S_SH // 512      # 3 s-chunks of 512
NT_FULL = SKV // 128   # 8 full t-tiles
T_REM = SKV - NT_FULL * 128  # 12
NT = NT_FULL + 1       # 9 t-tiles
NE = DE // 128         # 8
NCC = DC // 128        # 6
INV_SQRT_DH = 1.0 / math.sqrt(DH)

F32 = mybir.dt.float32
F32R = mybir.dt.float32r
BF16 = mybir.dt.bfloat16


def _t_width(ti):
    return 128 if ti < NT_FULL else T_REM


def build_bass():
    nc = bacc.Bacc("TRN2", target_bir_lowering=False, debug=False)

    xT_d = nc.dram_tensor("xT", [DE, S_SH], F32, kind="ExternalInput").ap()
    yT_d = nc.dram_tensor("yT", [DC, SKV], F32, kind="ExternalInput").ap()
    wqT_d = nc.dram_tensor("WqT", [DE, DE], F32, kind="ExternalInput").ap()
    wkT_d = nc.dram_tensor("WkT", [DC, DE], F32, kind="ExternalInput").ap()
    wvT_d = nc.dram_tensor("WvT", [DC, DE], F32, kind="ExternalInput").ap()
    woT_d = nc.dram_tensor("WoT", [DE, DE], F32, kind="ExternalInput").ap()
    bq_d = nc.dram_tensor("bq", [DE], F32, kind="ExternalInput").ap()
    bk_d = nc.dram_tensor("bk", [DE], F32, kind="ExternalInput").ap()
    cst_d = nc.dram_tensor("constb", [DE], F32, kind="ExternalInput").ap()
    out_d = nc.dram_tensor("out", [S_SH, DE], F32, kind="ExternalOutput").ap()

    with tile.TileContext(nc) as tc:
        with (
            tc.tile_pool(name="misc", bufs=1) as misc,
            tc.tile_pool(name="attn_keep", bufs=1) as attn_keep,
        ):
            ident_f = misc.tile([128, 128], F32)
            make_identity(nc, ident_f)
            bq_sb = misc.tile([128, NE], F32)
            nc.sync.dma_start(bq_sb, bq_d.rearrange("(j p) -> p j", p=128))
            bk_sb = misc.tile([128, NE], F32)
            nc.sync.dma_start(bk_sb, bk_d.rearrange("(j p) -> p j", p=128))
            cst_sb = misc.tile([128, NE], F32)
            nc.sync.dma_start(cst_sb, cst_d.rearrange("(j p) -> p j", p=128))

            attnT = attn_keep.tile([128, NE, S_SH], F32R)

            # ---- stage Q: qT[d, s] ----
            with tc.tile_pool(name="q_keep", bufs=1) as q_keep:
                qT = q_keep.tile([128, NE, S_SH], F32R)
                y_in = tc.alloc_tile_pool(name="y_in", bufs=1)
                yT = y_in.tile([128, NCC, SKV], F32R)
                for i in range(NCC):
                    nc.gpsimd.dma_start(yT[:, i], yT_d[ts(i, 128)])
                with (
                    tc.tile_pool(name="xq_in", bufs=1) as xq_in,
                    tc.tile_pool(name="ps_q", bufs=8, space="PSUM") as ps_q,
                ):
                    xT = xq_in.tile([128, NE, S_SH], F32R)
                    wqT = xq_in.tile([128, NE, DE], F32R)
                    for i in range(NE):
                        nc.gpsimd.dma_start(xT[:, i], xT_d[ts(i, 128)])
                        nc.gpsimd.dma_start(wqT[:, i], wqT_d[ts(i, 128)])
                    for di in range(NE):
                        for sc in range(NSC):
                            ps = ps_q.tile([128, 512], F32, tag="psq")
                            for ei in range(NE):
                                nc.tensor.matmul(
                                    ps,
                                    wqT[:, ei, ts(di, 128)],
                                    xT[:, ei, ts(sc, 512)],
                                    start=(ei == 0),
                                    stop=(ei == NE - 1),
                                )
                            nc.scalar.activation(
                                qT[:, di, ts(sc, 512)], ps,
                                mybir.ActivationFunctionType.Identity,
                                bias=bq_sb[:, ds(di, 1)],
                            )

                # ---- stage K/V ----
                kv_keep = tc.alloc_tile_pool(name="kv_keep", bufs=1)
                kT = kv_keep.tile([128, NE, SKV], F32R)
                v_sb = kv_keep.tile([128, NT, DE], BF16)
                if True:
                    with (
                        tc.tile_pool(name="wk_in", bufs=1) as wk_in,
                        tc.tile_pool(name="ps_k", bufs=8, space="PSUM") as ps_k,
                    ):
                        wkT = wk_in.tile([128, NCC, DE], F32R)
                        for i in range(NCC):
                            nc.gpsimd.dma_start(wkT[:, i], wkT_d[ts(i, 128)])
                        for di in range(NE):
                            for tc_i in range(3):
                                t0, tw = tc_i * 512, min(512, SKV - tc_i * 512)
                                ps = ps_k.tile([128, 512], F32, tag="psk")
                                for ci in range(NCC):
                                    nc.tensor.matmul(
                                        ps[:, :tw],
                                        wkT[:, ci, ts(di, 128)],
                                        yT[:, ci, ds(t0, tw)],
                                        start=(ci == 0),
                                        stop=(ci == NCC - 1),
                                    )
                                nc.scalar.activation(
                                    kT[:, di, ds(t0, tw)], ps[:, :tw],
                                    mybir.ActivationFunctionType.Identity,
                                    bias=bk_sb[:, ds(di, 1)],
                                )

                    with (
                        tc.tile_pool(name="wv_in", bufs=1) as wv_in,
                        tc.tile_pool(name="ps_v", bufs=8, space="PSUM") as ps_v,
                    ):
                        wvT = wv_in.tile([128, NCC, DE], F32R)
                        for i in range(NCC):
                            nc.gpsimd.dma_start(wvT[:, i], wvT_d[ts(i, 128)])
                        for dc in range(2):
                            for ti in range(NT):
                                tw = _t_width(ti)
                                ps = ps_v.tile([128, 512], F32, tag="psv")
                                for ci in range(NCC):
                                    nc.tensor.matmul(
                                        ps[:tw],
                                        yT[:, ci, ds(ti * 128, tw)],
                                        wvT[:, ci, ts(dc, 512)],
                                        start=(ci == 0),
                                        stop=(ci == NCC - 1),
                                    )
                                nc.vector.tensor_copy(
                                    v_sb[:tw, ti, ts(dc, 512)], ps[:tw]
                                )

                # ---- attention (transposed scores layout) ----
                with (
                    tc.tile_pool(name="awork", bufs=2) as awork,
                    tc.tile_pool(name="ps_s", bufs=2, space="PSUM") as ps_s,
                    tc.tile_pool(name="ps_s8", bufs=1, space="PSUM") as ps_s8,
                    tc.tile_pool(name="ps_o", bufs=3, space="PSUM") as ps_o,
                ):
                    def stage_scores(h, sc):
                        expT = awork.tile([128, NT, 512], BF16, tag="expT")
                        # score pairs share a 2-bank psum tile; one exp each
                        for pi in range(4):
                            ps = ps_s.tile([128, 2, 512], F32, tag="pss")
                            for j in range(2):
                                ti = pi * 2 + j
                                nc.tensor.matmul(
                                    ps[:, j],
                                    kT[:, h, ts(ti, 128)],
                                    qT[:, h, ts(sc, 512)],
                                    start=True,
                                    stop=True,
                                )
                            nc.scalar.activation(
                                expT[:, ts(pi, 2)], ps,
                                mybir.ActivationFunctionType.Exp,
                                scale=INV_SQRT_DH,
                            )
                        ps8 = ps_s8.tile([128, 512], F32, tag="ps8")
                        nc.tensor.matmul(
                            ps8[:T_REM],
                            kT[:, h, ds(NT_FULL * 128, T_REM)],
                            qT[:, h, ts(sc, 512)],
                            start=True,
                            stop=True,
                        )
                        nc.scalar.activation(
                            expT[:T_REM, NT_FULL], ps8[:T_REM],
                            mybir.ActivationFunctionType.Exp,
                            scale=INV_SQRT_DH,
                        )
                        return (expT,)

                    def stage_pv(h, sc, expT):
                        pso = ps_o.tile([128, 512], F32, tag="pso")
                        for ti in range(NT):
                            tw = _t_width(ti)
                            nc.tensor.matmul(
                                pso,
                                v_sb[:tw, ti, ds(h * 128, 128)],
                                expT[:tw, ti],
                                start=(ti == 0),
                                stop=(ti == NT - 1),
                            )
                        nc.vector.tensor_copy(attnT[:, h, ts(sc, 512)], pso)
                        s01 = awork.tile([128, 512], F32, tag="s01")
                        s23 = awork.tile([128, 512], F32, tag="s23")
                        s45 = awork.tile([128, 512], F32, tag="s45")
                        s67 = awork.tile([128, 512], F32, tag="s67")
                        nc.vector.tensor_tensor(s01, expT[:, 0], expT[:, 1], op=mybir.AluOpType.add)
                        nc.vector.tensor_tensor(s23, expT[:, 2], expT[:, 3], op=mybir.AluOpType.add)
                        nc.vector.tensor_tensor(s45, expT[:, 4], expT[:, 5], op=mybir.AluOpType.add)
                        nc.vector.tensor_tensor(s67, expT[:, 6], expT[:, 7], op=mybir.AluOpType.add)
                        nc.vector.tensor_tensor(s01, s01, s23, op=mybir.AluOpType.add)
                        nc.vector.tensor_tensor(s45, s45, s67, op=mybir.AluOpType.add)
                        nc.vector.tensor_tensor(s01, s01, s45, op=mybir.AluOpType.add)
                        nc.vector.tensor_tensor(s01[:T_REM], s01[:T_REM], expT[:T_REM, NT_FULL], op=mybir.AluOpType.add)
                        nc.gpsimd.partition_all_reduce(
                            s23, s01, channels=128,
                            reduce_op=bass_isa.ReduceOp.add,
                        )
                        nc.vector.reciprocal_approx_fast(s45, s23)
                        nc.vector.tensor_tensor(
                            attnT[:, h, ts(sc, 512)],
                            attnT[:, h, ts(sc, 512)], s45,
                            op=mybir.AluOpType.mult,
                        )

                    iters = [(h, sc) for h in range(H) for sc in range(NSC)]
                    prev = None
                    for h, sc in iters:
                        cur = (h, sc, *stage_scores(h, sc))
                        if prev is not None:
                            stage_pv(*prev)
                        prev = cur
                    stage_pv(*prev)
                kv_keep.release()
                y_in.release()

            # ---- output projection + final transpose ----
            with (
                tc.tile_pool(name="o_in", bufs=1) as o_in,
                tc.tile_pool(name="o_out", bufs=3) as o_out,
                tc.tile_pool(name="ps_f", bufs=4, space="PSUM") as ps_f,
                tc.tile_pool(name="ps_ft", bufs=2, space="PSUM") as ps_ft,
            ):
                woT = o_in.tile([128, NE, DE], F32R)
                for i in range(NE):
                    nc.gpsimd.dma_start(woT[:, i], woT_d[ts(i, 128)])
                finalT = o_in.tile([128, NE, S_SH], F32)
                for ei in range(NE):
                    for sc in range(NSC):
                        ps = ps_f.tile([128, 512], F32, tag="psf")
                        for di in range(NE):
                            nc.tensor.matmul(
                                ps,
                                woT[:, di, ts(ei, 128)],
                                attnT[:, di, ts(sc, 512)],
                                start=(di == 0),
                                stop=(di == NE - 1),
                            )
                        nc.scalar.activation(
                            finalT[:, ei, ts(sc, 512)], ps,
                            mybir.ActivationFunctionType.Identity,
                            bias=cst_sb[:, ds(ei, 1)],
                        )
                for si in range(NS):
                    out_sb = o_out.tile([128, DE], F32, tag="osb")
                    for g in range(2):
                        pst = ps_ft.tile([128, 512], F32, tag="psft")
                        for j in range(4):
                            nc.tensor.transpose(
                                pst[:, ts(j, 128)],
                                finalT[:, g * 4 + j, ts(si, 128)],
                                ident_f,
                            )
                        nc.vector.tensor_copy(out_sb[:, ts(g, 512)], pst)
                    nc.sync.dma_start(out_d[ts(si, 128)], out_sb)

    nc.compile()
    return nc


_NC_CACHE = None


def _get_nc():
    global _NC_CACHE
    if _NC_CACHE is None:
        _NC_CACHE = build_bass()
    return _NC_CACHE


def make_in_maps(inputs):
    x = np.asarray(inputs["x"], np.float32)
    y = np.asarray(inputs["y"], np.float32)
    Wq = np.asarray(inputs["Wq"], np.float32)
    Wk = np.asarray(inputs["Wk"], np.float32)
    Wv = np.asarray(inputs["Wv"], np.float32)
    Wo = np.asarray(inputs["Wo"], np.float32)
    bq = np.asarray(inputs["bq"], np.float32)
    bk = np.asarray(inputs["bk"], np.float32)
    bv = np.asarray(inputs["bv"], np.float32)
    bo = np.asarray(inputs["bo"], np.float32)

    shared = {
        "WqT": np.ascontiguousarray(Wq.T),
        "WkT": np.ascontiguousarray(Wk.T),
        "WvT": np.ascontiguousarray(Wv.T),
        "WoT": np.ascontiguousarray(Wo.T),
        "bq": bq,
        "bk": bk,
        "constb": (Wo @ bv + bo).astype(np.float32),
    }
    in_maps = []
    for c in range(N_CORES):
        b, sh = c // 2, c % 2
        in_maps.append(
            {
                "xT": np.ascontiguousarray(x[b, sh * S_SH : (sh + 1) * S_SH].T),
                "yT": np.ascontiguousarray(y[b].T),
                **shared,
            }
        )
    return in_maps


def gather(results):
    out = np.empty((B, S, DE), np.float32)
    for c in range(N_CORES):
        b, sh = c // 2, c % 2
        out[b, sh * S_SH : (sh + 1) * S_SH] = results[c]["out"]
    return out


def kernel(**inputs) -> np.ndarray:
    nc = _get_nc()
    in_maps = make_in_maps(inputs)
    res = run_bass_kernel_spmd(nc, in_maps, core_ids=list(range(N_CORES)))
    return gather(res.results)


# revision 15
# speedup vs baseline: 1.4475x; 1.1938x over previous
"""Cross-attention Trainium2 kernel (8 NeuronCores, SPMD).

Problem: B=4, S=3072, SKV=1036, D_EMBED=1024, D_CROSS=768, H=8, d_head=128.
Sharding: core c -> (batch b = c//2, query-row half sh = c%2). Each core
computes its full [1536, 1024] output slice; gather is pure concatenation.

Per-core device program (all tensors feature-on-partition, token-on-free):
  qT[d,s]  = WqT.T @ xT    (fp32r matmuls, bias bq via ACT-copy bias)
  kT[d,t]  = WkT.T @ yT    (bias bk via ACT-copy bias)
  v[t,d]   = yT.T @ WvT    (-> bf16; bias bv folded into output const)
  per head h, per 512-col s-chunk:
    scores[s,t] = qT_h_slice.T @ kT_h      (psum, fp32r)
    p = exp(scores * 1/sqrt(128))          (ACT, accum_out -> row sums)
    p *= 1/rowsum                          (DVE tensor_scalar, bf16)
    pT = transpose(p)                      (PE transposes, bf16)
    attnT_h[:, sc] = sum_t v_h[t].T @ pT[t]  (psum accumulate)
  finalT[e,s] = WoT.T @ attnT + const      (fp32r; const = Wo@bv + bo)
  out[s,e]   = transpose(finalT)           (PE transposes, fp32r)
"""

import sys

sys.path.insert(0, "/opt/trn_rl_repo")

import math

import numpy as np

import concourse.bass as bass
import concourse.mybir as mybir
import concourse.tile as tile
from concourse import bacc
from concourse.bass import ts, ds
from concourse.bass_utils import run_bass_kernel_spmd
from concourse.masks import make_identity
from concourse import bass_isa

N_CORES = 8
B, S, SKV = 4, 3072, 1036
DE, DC, H, DH = 1024, 768, 8, 128
S_SH = S // 2          # 1536 query rows per core
NS = S_SH // 128       # 12 s-tiles
NSC = S_SH // 512      # 3 s-chunks of 512
NT_FULL = SKV // 128   # 8 full t-tiles
T_REM = SKV - NT_FULL * 128  # 12
NT = NT_FULL + 1       # 9 t-tiles
NE = DE // 128         # 8
NCC = DC // 128        # 6
INV_SQRT_DH = 1.0 / math.sqrt(DH)

F32 = mybir.dt.float32
F32R = mybir.dt.float32r
BF16 = mybir.dt.bfloat16


def _t_width(ti):
    return 128 if ti < NT_FULL else T_REM


def build_bass():
    nc = bacc.Bacc("TRN2", target_bir_lowering=False, debug=False)

    xT_d = nc.dram_tensor("xT", [DE, S_SH], F32, kind="ExternalInput").ap()
    yT_d = nc.dram_tensor("yT", [DC, SKV], F32, kind="ExternalInput").ap()
    wqT_d = nc.dram_tensor("WqT", [DE, DE], F32, kind="ExternalInput").ap()
    wkT_d = nc.dram_tensor("WkT", [DC, DE], F32, kind="ExternalInput").ap()
    wvT_d = nc.dram_tensor("WvT", [DC, DE], F32, kind="ExternalInput").ap()
    woT_d = nc.dram_tensor("WoT", [DE, DE], F32, kind="ExternalInput").ap()
    bq_d = nc.dram_tensor("bq", [DE], F32, kind="ExternalInput").ap()
    bk_d = nc.dram_tensor("bk", [DE], F32, kind="ExternalInput").ap()
    cst_d = nc.dram_tensor("constb", [DE], F32, kind="ExternalInput").ap()
    out_d = nc.dram_tensor("out", [S_SH, DE], F32, kind="ExternalOutput").ap()

    with tile.TileContext(nc) as tc:
        with (
            tc.tile_pool(name="misc", bufs=1) as misc,
            tc.tile_pool(name="attn_keep", bufs=1) as attn_keep,
        ):
            ident_f = misc.tile([128, 128], F32)
            make_identity(nc, ident_f)
            bq_sb = misc.tile([128, NE], F32)
            nc.sync.dma_start(bq_sb, bq_d.rearrange("(j p) -> p j", p=128))
            bk_sb = misc.tile([128, NE], F32)
            nc.sync.dma_start(bk_sb, bk_d.rearrange("(j p) -> p j", p=128))
            cst_sb = misc.tile([128, NE], F32)
            nc.sync.dma_start(cst_sb, cst_d.rearrange("(j p) -> p j", p=128))

            attnT = attn_keep.tile([128, NE, S_SH], F32R)

            # ---- stage Q: qT[d, s] (xT streamed per 512-col chunk) ----
            with tc.tile_pool(name="q_keep", bufs=1) as q_keep:
                qT = q_keep.tile([128, NE, S_SH], F32R)
                kv_keep = tc.alloc_tile_pool(name="kv_keep", bufs=1)
                kT = kv_keep.tile([128, NE, SKV], F32R)
                v_sb = kv_keep.tile([128, NT, DE], BF16)
                with (
                    tc.tile_pool(name="xq_in", bufs=1) as xq_in,
                    tc.tile_pool(name="xch_in", bufs=2) as xch_in,
                    tc.tile_pool(name="ps_q", bufs=8, space="PSUM") as ps_q,
                ):
                    wqT = xq_in.tile([128, NE, DE], F32R)
                    for i in range(NE):
                        nc.gpsimd.dma_start(wqT[:, i], wqT_d[ts(i, 128)])
                    for sc in range(4):
                        xch = xch_in.tile([128, NE, 384], F32R, tag="xch")
                        for i in range(NE):
                            nc.gpsimd.dma_start(
                                xch[:, i], xT_d[ts(i, 128), ts(sc, 384)]
                            )
                        for di in range(NE):
                            ps = ps_q.tile([128, 384], F32, tag="psq")
                            for ei in range(NE):
                                nc.tensor.matmul(
                                    ps,
                                    wqT[:, ei, ts(di, 128)],
                                    xch[:, ei],
                                    start=(ei == 0),
                                    stop=(ei == NE - 1),
                                )
                            nc.scalar.activation(
                                qT[:, di, ts(sc, 384)], ps,
                                mybir.ActivationFunctionType.Identity,
                                bias=bq_sb[:, ds(di, 1)],
                            )

                # ---- stage K/V ----
                y_in = tc.alloc_tile_pool(name="y_in", bufs=1)
                yT = y_in.tile([128, NCC, SKV], F32R)
                for i in range(NCC):
                    nc.gpsimd.dma_start(yT[:, i], yT_d[ts(i, 128)])
                if True:
                    with (
                        tc.tile_pool(name="wk_in", bufs=1) as wk_in,
                        tc.tile_pool(name="ps_k", bufs=8, space="PSUM") as ps_k,
                    ):
                        wkT = wk_in.tile([128, NCC, DE], F32R)
                        for i in range(NCC):
                            nc.gpsimd.dma_start(wkT[:, i], wkT_d[ts(i, 128)])
                        for di in range(NE):
                            for tc_i in range(3):
                                t0, tw = tc_i * 512, min(512, SKV - tc_i * 512)
                                ps = ps_k.tile([128, 512], F32, tag="psk")
                                for ci in range(NCC):
                                    nc.tensor.matmul(
                                        ps[:, :tw],
                                        wkT[:, ci, ts(di, 128)],
                                        yT[:, ci, ds(t0, tw)],
                                        start=(ci == 0),
                                        stop=(ci == NCC - 1),
                                    )
                                nc.scalar.activation(
                                    kT[:, di, ds(t0, tw)], ps[:, :tw],
                                    mybir.ActivationFunctionType.Identity,
                                    bias=bk_sb[:, ds(di, 1)],
                                )

                    with (
                        tc.tile_pool(name="wv_in", bufs=1) as wv_in,
                        tc.tile_pool(name="ps_v", bufs=8, space="PSUM") as ps_v,
                    ):
                        wvT = wv_in.tile([128, NCC, DE], F32R)
                        for i in range(NCC):
                            nc.gpsimd.dma_start(wvT[:, i], wvT_d[ts(i, 128)])
                        for dc in range(2):
                            for ti in range(NT):
                                tw = _t_width(ti)
                                ps = ps_v.tile([128, 512], F32, tag="psv")
                                for ci in range(NCC):
                                    nc.tensor.matmul(
                                        ps[:tw],
                                        yT[:, ci, ds(ti * 128, tw)],
                                        wvT[:, ci, ts(dc, 512)],
                                        start=(ci == 0),
                                        stop=(ci == NCC - 1),
                                    )
                                nc.vector.tensor_copy(
                                    v_sb[:tw, ti, ts(dc, 512)], ps[:tw]
                                )
                    y_in.release()

                # ---- attention: scoresT -> exp -> PV + Z (all PE) ----
                with (
                    tc.tile_pool(name="awork", bufs=2) as awork,
                    tc.tile_pool(name="ps_s", bufs=2, space="PSUM") as ps_s,
                    tc.tile_pool(name="ps_o", bufs=2, space="PSUM") as ps_o,
                    tc.tile_pool(name="ps_z", bufs=2, space="PSUM") as ps_z,
                ):
                    ones_sb = misc.tile([128, 128], BF16)
                    nc.any.memset(ones_sb, 1.0)

                    def stage_scores(h, sc):
                        expT = awork.tile([128, NT, 512], BF16, tag="expT")
                        for pi in range(5):
                            ps = ps_s.tile([128, 2, 512], F32, tag="pss")
                            nj = 2 if pi < 4 else 1
                            for j in range(nj):
                                ti = pi * 2 + j
                                tw = _t_width(ti)
                                nc.tensor.matmul(
                                    ps[:tw, j],
                                    kT[:, h, ds(ti * 128, tw)],
                                    qT[:, h, ts(sc, 512)],
                                    start=True,
                                    stop=True,
                                )
                            if nj == 2:
                                nc.scalar.activation(
                                    expT[:, ts(pi, 2)], ps,
                                    mybir.ActivationFunctionType.Exp,
                                    scale=INV_SQRT_DH,
                                )
                            else:
                                nc.scalar.activation(
                                    expT[:T_REM, NT_FULL], ps[:T_REM, 0],
                                    mybir.ActivationFunctionType.Exp,
                                    scale=INV_SQRT_DH,
                                )
                        return (expT,)

                    def stage_pv(h, sc, expT):
                        pso = ps_o.tile([128, 512], F32, tag="pso")
                        zf = ps_z.tile([128, 512], F32, tag="zf")
                        for ti in range(NT):
                            tw = _t_width(ti)
                            nc.tensor.matmul(
                                pso,
                                v_sb[:tw, ti, ds(h * 128, 128)],
                                expT[:tw, ti],
                                start=(ti == 0),
                                stop=(ti == NT - 1),
                            )
                        for ti in range(NT):
                            tw = _t_width(ti)
                            nc.tensor.matmul(
                                zf,
                                ones_sb[:tw],
                                expT[:tw, ti],
                                start=(ti == 0),
                                stop=(ti == NT - 1),
                            )
                        nc.scalar.activation(
                            attnT[:, h, ts(sc, 512)], pso,
                            mybir.ActivationFunctionType.Copy,
                        )
                        return zf

                    def stage_norm(h, sc, zf):
                        zrb = awork.tile([128, 512], F32, tag="zrb")
                        nc.vector.reciprocal_approx_fast(zrb, zf)
                        nc.vector.tensor_tensor(
                            attnT[:, h, ts(sc, 512)],
                            attnT[:, h, ts(sc, 512)], zrb,
                            op=mybir.AluOpType.mult,
                        )

                    iters = [(h, sc) for h in range(H) for sc in range(NSC)]
                    pipeA = None  # (h, sc, expT)
                    pipeB = None  # (h, sc, zf)
                    for h, sc in iters:
                        curA = (h, sc, *stage_scores(h, sc))
                        if pipeB is not None:
                            stage_norm(*pipeB)
                            pipeB = None
                        if pipeA is not None:
                            pipeB = (pipeA[0], pipeA[1], stage_pv(*pipeA))
                        pipeA = curA
                    pipeB2 = (pipeA[0], pipeA[1], stage_pv(*pipeA))
                    stage_norm(*pipeB)
                    stage_norm(*pipeB2)
                kv_keep.release()

            # ---- output projection + final transpose ----
            with (
                tc.tile_pool(name="o_in", bufs=1) as o_in,
                tc.tile_pool(name="o_out", bufs=3) as o_out,
                tc.tile_pool(name="ps_f", bufs=4, space="PSUM") as ps_f,
                tc.tile_pool(name="ps_ft", bufs=2, space="PSUM") as ps_ft,
            ):
                woT = o_in.tile([128, NE, DE], F32R)
                for i in range(NE):
                    nc.gpsimd.dma_start(woT[:, i], woT_d[ts(i, 128)])
                finalT = o_in.tile([128, NE, S_SH], F32)
                for ei in range(NE):
                    for sc in range(NSC):
                        ps = ps_f.tile([128, 512], F32, tag="psf")
                        for di in range(NE):
                            nc.tensor.matmul(
                                ps,
                                woT[:, di, ts(ei, 128)],
                                attnT[:, di, ts(sc, 512)],
                                start=(di == 0),
                                stop=(di == NE - 1),
                            )
                        nc.scalar.activation(
                            finalT[:, ei, ts(sc, 512)], ps,
                            mybir.ActivationFunctionType.Identity,
                            bias=cst_sb[:, ds(ei, 1)],
                        )
                for si in range(NS):
                    out_sb = o_out.tile([128, DE], F32, tag="osb")
                    for g in range(2):
                        pst = ps_ft.tile([128, 512], F32, tag="psft")
                        for j in range(4):
                            nc.tensor.transpose(
                                pst[:, ts(j, 128)],
                                finalT[:, g * 4 + j, ts(si, 128)],
                                ident_f,
                            )
                        nc.vector.tensor_copy(out_sb[:, ts(g, 512)], pst)
                    nc.sync.dma_start(out_d[ts(si, 128)], out_sb)

    nc.compile()
    return nc


_NC_CACHE = None


def _get_nc():
    global _NC_CACHE
    if _NC_CACHE is None:
        _NC_CACHE = build_bass()
    return _NC_CACHE


def make_in_maps(inputs):
    x = np.asarray(inputs["x"], np.float32)
    y = np.asarray(inputs["y"], np.float32)
    Wq = np.asarray(inputs["Wq"], np.float32)
    Wk = np.asarray(inputs["Wk"], np.float32)
    Wv = np.asarray(inputs["Wv"], np.float32)
    Wo = np.asarray(inputs["Wo"], np.float32)
    bq = np.asarray(inputs["bq"], np.float32)
    bk = np.asarray(inputs["bk"], np.float32)
    bv = np.asarray(inputs["bv"], np.float32)
    bo = np.asarray(inputs["bo"], np.float32)

    shared = {
        "WqT": np.ascontiguousarray(Wq.T),
        "WkT": np.ascontiguousarray(Wk.T),
        "WvT": np.ascontiguousarray(Wv.T),
        "WoT": np.ascontiguousarray(Wo.T),
        "bq": bq,
        "bk": bk,
        "constb": (Wo @ bv + bo).astype(np.float32),
    }
    in_maps = []
    for c in range(N_CORES):
        b, sh = c // 2, c % 2
        in_maps.append(
            {
                "xT": np.ascontiguousarray(x[b, sh * S_SH : (sh + 1) * S_SH].T),
                "yT": np.ascontiguousarray(y[b].T),
                **shared,
            }
        )
    return in_maps


def gather(results):
    out = np.empty((B, S, DE), np.float32)
    for c in range(N_CORES):
        b, sh = c // 2, c % 2
        out[b, sh * S_SH : (sh + 1) * S_SH] = results[c]["out"]
    return out


def kernel(**inputs) -> np.ndarray:
    nc = _get_nc()
    in_maps = make_in_maps(inputs)
    res = run_bass_kernel_spmd(nc, in_maps, core_ids=list(range(N_CORES)))
    return gather(res.results)


# revision 16
# speedup vs baseline: 1.5468x; 1.0686x over previous
"""Cross-attention Trainium2 kernel (8 NeuronCores, SPMD).

Problem: B=4, S=3072, SKV=1036, D_EMBED=1024, D_CROSS=768, H=8, d_head=128.
Sharding: core c -> (batch b = c//2, query-row half sh = c%2). Each core
computes its full [1536, 1024] output slice; gather is pure concatenation.

Per-core device program (all tensors feature-on-partition, token-on-free):
  qT[d,s]  = WqT.T @ xT    (fp32r matmuls, bias bq via ACT-copy bias)
  kT[d,t]  = WkT.T @ yT    (bias bk via ACT-copy bias)
  v[t,d]   = yT.T @ WvT    (-> bf16; bias bv folded into output const)
  per head h, per 512-col s-chunk:
    scores[s,t] = qT_h_slice.T @ kT_h      (psum, fp32r)
    p = exp(scores * 1/sqrt(128))          (ACT, accum_out -> row sums)
    p *= 1/rowsum                          (DVE tensor_scalar, bf16)
    pT = transpose(p)                      (PE transposes, bf16)
    attnT_h[:, sc] = sum_t v_h[t].T @ pT[t]  (psum accumulate)
  finalT[e,s] = WoT.T @ attnT + const      (fp32r; const = Wo@bv + bo)
  out[s,e]   = transpose(finalT)           (PE transposes, fp32r)
"""

import sys

sys.path.insert(0, "/opt/trn_rl_repo")

import math

import numpy as np

import concourse.bass as bass
import concourse.mybir as mybir
import concourse.tile as tile
from concourse import bacc
from concourse.bass import ts, ds
from concourse.bass_utils import run_bass_kernel_spmd
from concourse.masks import make_identity
from concourse import bass_isa

N_CORES = 8
B, S, SKV = 4, 3072, 1036
DE, DC, H, DH = 1024, 768, 8, 128
S_SH = S // 2          # 1536 query rows per core
NS = S_SH // 128       # 12 s-tiles
NSC = S_SH // 512      # 3 s-chunks of 512
NT_FULL = SKV // 128   # 8 full t-tiles
T_REM = SKV - NT_FULL * 128  # 12
NT = NT_FULL + 1       # 9 t-tiles
NE = DE // 128         # 8
NCC = DC // 128        # 6
INV_SQRT_DH = 1.0 / math.sqrt(DH)

F32 = mybir.dt.float32
F32R = mybir.dt.float32r
BF16 = mybir.dt.bfloat16


def _t_width(ti):
    return 128 if ti < NT_FULL else T_REM


def build_bass():
    nc = bacc.Bacc("TRN2", target_bir_lowering=False, debug=False)

    xT_d = nc.dram_tensor("xT", [DE, S_SH], F32, kind="ExternalInput").ap()
    yT_d = nc.dram_tensor("yT", [DC, SKV], F32, kind="ExternalInput").ap()
    wqT_d = nc.dram_tensor("WqT", [DE, DE], F32, kind="ExternalInput").ap()
    wkT_d = nc.dram_tensor("WkT", [DC, DE], F32, kind="ExternalInput").ap()
    wvT_d = nc.dram_tensor("WvT", [DC, DE], F32, kind="ExternalInput").ap()
    woT_d = nc.dram_tensor("WoT", [DE, DE], F32, kind="ExternalInput").ap()
    bq_d = nc.dram_tensor("bq", [DE], F32, kind="ExternalInput").ap()
    bk_d = nc.dram_tensor("bk", [DE], F32, kind="ExternalInput").ap()
    cst_d = nc.dram_tensor("constb", [DE], F32, kind="ExternalInput").ap()
    out_d = nc.dram_tensor("out", [S_SH, DE], F32, kind="ExternalOutput").ap()

    with tile.TileContext(nc) as tc:
        with (
            tc.tile_pool(name="misc", bufs=1) as misc,
            tc.tile_pool(name="attn_keep", bufs=1) as attn_keep,
        ):
            ident_f = misc.tile([128, 128], F32)
            make_identity(nc, ident_f)
            ones_sb = misc.tile([128, 128], BF16)
            nc.any.memset(ones_sb, 1.0)
            bq_sb = misc.tile([128, NE], F32)
            nc.sync.dma_start(bq_sb, bq_d.rearrange("(j p) -> p j", p=128))
            bk_sb = misc.tile([128, NE], F32)
            nc.sync.dma_start(bk_sb, bk_d.rearrange("(j p) -> p j", p=128))
            cst_sb = misc.tile([128, NE], F32)
            nc.sync.dma_start(cst_sb, cst_d.rearrange("(j p) -> p j", p=128))

            attnT = attn_keep.tile([128, NE, S_SH], F32R)

            with tc.tile_pool(name="q_keep", bufs=1) as q_keep:
                qT = q_keep.tile([128, NE, S_SH], BF16)
                kv_keep = tc.alloc_tile_pool(name="kv_keep", bufs=1)
                kT = kv_keep.tile([128, NE, SKV], BF16)
                v_sb = kv_keep.tile([128, NT, DE], BF16)

                # all weights + yT prefetched up front (bf16 casting DMAs)
                w_in = tc.alloc_tile_pool(name="w_in", bufs=1)
                wqT = w_in.tile([128, NE, DE], BF16)
                yT = w_in.tile([128, NCC, SKV], BF16)
                wkT = w_in.tile([128, NCC, DE], BF16)
                wvT = w_in.tile([128, NCC, DE], BF16)
                for i in range(NE):
                    nc.gpsimd.dma_start(wqT[:, i], wqT_d[ts(i, 128)])
                for i in range(NCC):
                    nc.gpsimd.dma_start(yT[:, i], yT_d[ts(i, 128)])
                    nc.gpsimd.dma_start(wkT[:, i], wkT_d[ts(i, 128)])
                    nc.gpsimd.dma_start(wvT[:, i], wvT_d[ts(i, 128)])

                # ---- stage Q (xT streamed per 384-col chunk, bf16) ----
                with (
                    tc.tile_pool(name="xch_in", bufs=2) as xch_in,
                    tc.tile_pool(name="ps_q", bufs=8, space="PSUM") as ps_q,
                ):
                    for sc in range(4):
                        xch = xch_in.tile([128, NE, 384], BF16, tag="xch")
                        for i in range(NE):
                            nc.gpsimd.dma_start(
                                xch[:, i], xT_d[ts(i, 128), ts(sc, 384)]
                            )
                        for di in range(NE):
                            ps = ps_q.tile([128, 384], F32, tag="psq")
                            for ei in range(NE):
                                nc.tensor.matmul(
                                    ps,
                                    wqT[:, ei, ts(di, 128)],
                                    xch[:, ei],
                                    start=(ei == 0),
                                    stop=(ei == NE - 1),
                                )
                            nc.scalar.activation(
                                qT[:, di, ts(sc, 384)], ps,
                                mybir.ActivationFunctionType.Identity,
                                bias=bq_sb[:, ds(di, 1)],
                            )

                # ---- stage K ----
                with tc.tile_pool(name="ps_k", bufs=8, space="PSUM") as ps_k:
                    for di in range(NE):
                        for tc_i in range(3):
                            t0, tw = tc_i * 512, min(512, SKV - tc_i * 512)
                            ps = ps_k.tile([128, 512], F32, tag="psk")
                            for ci in range(NCC):
                                nc.tensor.matmul(
                                    ps[:, :tw],
                                    wkT[:, ci, ts(di, 128)],
                                    yT[:, ci, ds(t0, tw)],
                                    start=(ci == 0),
                                    stop=(ci == NCC - 1),
                                )
                            nc.scalar.activation(
                                kT[:, di, ds(t0, tw)], ps[:, :tw],
                                mybir.ActivationFunctionType.Identity,
                                bias=bk_sb[:, ds(di, 1)],
                            )

                # ---- stage V ----
                with tc.tile_pool(name="ps_v", bufs=8, space="PSUM") as ps_v:
                    for dc in range(2):
                        for ti in range(NT):
                            tw = _t_width(ti)
                            ps = ps_v.tile([128, 512], F32, tag="psv")
                            for ci in range(NCC):
                                nc.tensor.matmul(
                                    ps[:tw],
                                    yT[:, ci, ds(ti * 128, tw)],
                                    wvT[:, ci, ts(dc, 512)],
                                    start=(ci == 0),
                                    stop=(ci == NCC - 1),
                                )
                            nc.vector.tensor_copy(
                                v_sb[:tw, ti, ts(dc, 512)], ps[:tw]
                            )

                # ---- attention: scoresT -> exp -> PV + Z (all PE) ----
                with (
                    tc.tile_pool(name="awork", bufs=2) as awork,
                    tc.tile_pool(name="ps_s", bufs=2, space="PSUM") as ps_s,
                    tc.tile_pool(name="ps_o", bufs=2, space="PSUM") as ps_o,
                    tc.tile_pool(name="ps_z", bufs=2, space="PSUM") as ps_z,
                ):
                    def stage_scores(h, sc):
                        expT = awork.tile([128, NT, 512], BF16, tag="expT")
                        for pi in range(5):
                            ps = ps_s.tile([128, 2, 512], F32, tag="pss")
                            nj = 2 if pi < 4 else 1
                            for j in range(nj):
                                ti = pi * 2 + j
                                tw = _t_width(ti)
                                nc.tensor.matmul(
                                    ps[:tw, j],
                                    kT[:, h, ds(ti * 128, tw)],
                                    qT[:, h, ts(sc, 512)],
                                    start=True,
                                    stop=True,
                                )
                            if nj == 2:
                                nc.scalar.activation(
                                    expT[:, ts(pi, 2)], ps,
                                    mybir.ActivationFunctionType.Exp,
                                    scale=INV_SQRT_DH,
                                )
                            else:
                                nc.scalar.activation(
                                    expT[:T_REM, NT_FULL], ps[:T_REM, 0],
                                    mybir.ActivationFunctionType.Exp,
                                    scale=INV_SQRT_DH,
                                )
                        return (expT,)

                    def stage_pv(h, sc, expT):
                        pso = ps_o.tile([128, 512], F32, tag="pso")
                        zf = ps_z.tile([128, 512], F32, tag="zf")
                        for ti in range(NT):
                            tw = _t_width(ti)
                            nc.tensor.matmul(
                                pso,
                                v_sb[:tw, ti, ds(h * 128, 128)],
                                expT[:tw, ti],
                                start=(ti == 0),
                                stop=(ti == NT - 1),
                            )
                        for ti in range(NT):
                            tw = _t_width(ti)
                            nc.tensor.matmul(
                                zf,
                                ones_sb[:tw],
                                expT[:tw, ti],
                                start=(ti == 0),
                                stop=(ti == NT - 1),
                            )
                        nc.scalar.activation(
                            attnT[:, h, ts(sc, 512)], pso,
                            mybir.ActivationFunctionType.Copy,
                        )
                        return zf

                    def stage_norm(h, sc, zf):
                        zrb = awork.tile([128, 512], F32, tag="zrb")
                        nc.vector.reciprocal_approx_fast(zrb, zf)
                        nc.vector.tensor_tensor(
                            attnT[:, h, ts(sc, 512)],
                            attnT[:, h, ts(sc, 512)], zrb,
                            op=mybir.AluOpType.mult,
                        )

                    iters = [(h, sc) for h in range(H) for sc in range(NSC)]
                    pipeA = None
                    pipeB = None
                    for h, sc in iters:
                        curA = (h, sc, *stage_scores(h, sc))
                        if pipeB is not None:
                            stage_norm(*pipeB)
                            pipeB = None
                        if pipeA is not None:
                            pipeB = (pipeA[0], pipeA[1], stage_pv(*pipeA))
                        pipeA = curA
                    pipeB2 = (pipeA[0], pipeA[1], stage_pv(*pipeA))
                    stage_norm(*pipeB)
                    stage_norm(*pipeB2)
                w_in.release()
                kv_keep.release()

            # ---- output projection + final transpose (sc-outer) ----
            with (
                tc.tile_pool(name="o_in", bufs=1) as o_in,
                tc.tile_pool(name="o_out", bufs=3) as o_out,
                tc.tile_pool(name="ps_f", bufs=4, space="PSUM") as ps_f,
                tc.tile_pool(name="ps_ft", bufs=2, space="PSUM") as ps_ft,
            ):
                woT = o_in.tile([128, NE, DE], F32R)
                for i in range(NE):
                    nc.gpsimd.dma_start(woT[:, i], woT_d[ts(i, 128)])
                finalT = o_in.tile([128, NE, S_SH], F32)
                for sc in range(NSC):
                    for ei in range(NE):
                        ps = ps_f.tile([128, 512], F32, tag="psf")
                        for di in range(NE):
                            nc.tensor.matmul(
                                ps,
                                woT[:, di, ts(ei, 128)],
                                attnT[:, di, ts(sc, 512)],
                                start=(di == 0),
                                stop=(di == NE - 1),
                            )
                        nc.scalar.activation(
                            finalT[:, ei, ts(sc, 512)], ps,
                            mybir.ActivationFunctionType.Identity,
                            bias=cst_sb[:, ds(ei, 1)],
                        )
                    for sj in range(4):
                        si = sc * 4 + sj
                        out_sb = o_out.tile([128, DE], F32, tag="osb")
                        for g in range(2):
                            pst = ps_ft.tile([128, 512], F32, tag="psft")
                            for j in range(4):
                                nc.tensor.transpose(
                                    pst[:, ts(j, 128)],
                                    finalT[:, g * 4 + j, ts(si, 128)],
                                    ident_f,
                                )
                            nc.vector.tensor_copy(out_sb[:, ts(g, 512)], pst)
                        nc.sync.dma_start(out_d[ts(si, 128)], out_sb)

    nc.compile()
    return nc


_NC_CACHE = None


def _get_nc():
    global _NC_CACHE
    if _NC_CACHE is None:
        _NC_CACHE = build_bass()
    return _NC_CACHE


def make_in_maps(inputs):
    x = np.asarray(inputs["x"], np.float32)
    y = np.asarray(inputs["y"], np.float32)
    Wq = np.asarray(inputs["Wq"], np.float32)
    Wk = np.asarray(inputs["Wk"], np.float32)
    Wv = np.asarray(inputs["Wv"], np.float32)
    Wo = np.asarray(inputs["Wo"], np.float32)
    bq = np.asarray(inputs["bq"], np.float32)
    bk = np.asarray(inputs["bk"], np.float32)
    bv = np.asarray(inputs["bv"], np.float32)
    bo = np.asarray(inputs["bo"], np.float32)

    shared = {
        "WqT": np.ascontiguousarray(Wq.T),
        "WkT": np.ascontiguousarray(Wk.T),
        "WvT": np.ascontiguousarray(Wv.T),
        "WoT": np.ascontiguousarray(Wo.T),
        "bq": bq,
        "bk": bk,
        "constb": (Wo @ bv + bo).astype(np.float32),
    }
    in_maps = []
    for c in range(N_CORES):
        b, sh = c // 2, c % 2
        in_maps.append(
            {
                "xT": np.ascontiguousarray(x[b, sh * S_SH : (sh + 1) * S_SH].T),
                "yT": np.ascontiguousarray(y[b].T),
                **shared,
            }
        )
    return in_maps


def gather(results):
    out = np.empty((B, S, DE), np.float32)
    for c in range(N_CORES):
        b, sh = c // 2, c % 2
        out[b, sh * S_SH : (sh + 1) * S_SH] = results[c]["out"]
    return out


def kernel(**inputs) -> np.ndarray:
    nc = _get_nc()
    in_maps = make_in_maps(inputs)
    res = run_bass_kernel_spmd(nc, in_maps, core_ids=list(range(N_CORES)))
    return gather(res.results)
